# revision 31
# baseline (speedup 1.0000x reference)
"""Trainium2 Bass kernel for nn_MultiHeadAttn (unnormalized spatial attention).

Reference computation (per sample s of B=16):
    X = a[s]               # [C=256, HW=4096]  (H=64 rows of W=64)
    QT = wq @ X + bq       # [C, HW]   (q channels on rows)
    KT = wk @ X + bk
    V  = (wv @ X + bv).T   # [HW, C]   (hw on rows)
    per h: attnT_h = K_h @ Q_h^T        # [W, W]  == (Q_h K_h^T)^T
           attoutT_h = V_h^T @ attnT_h  # [C, W]
    out[s] = a[s] + attoutT (reassembled [C, HW])

Sharding: data-parallel over batch, 2 samples per core on 8 cores.
All matmuls in bf16 (fp32 PSUM accumulation); residual in fp32 PSUM +
bf16 operand, stored as bf16 and widened to f32 on the host.

Schedule: 16 global groups (2 samples x 8 column blocks of 512 hw
positions) run through a 2-stage software pipeline so the PE never
waits on the ACT engine:
    iteration i: projections(i) | QK^T(i-1) + extract | attout(i-2)
Input loads ride the ACT hardware DGE queue, stores the SP queue, so
sample 1's loads never sit behind sample 0's output stores.
"""

import numpy as np
import ml_dtypes

import concourse.bass as bass
import concourse.mybir as mybir
import concourse.tile as tile
from concourse import bacc
from concourse.bass_utils import run_bass_kernel_spmd

BF16 = mybir.dt.bfloat16
F32 = mybir.dt.float32
AF = mybir.ActivationFunctionType

N_CORES = 8
B, C, H, W = 16, 256, 64, 64
HW = H * W               # 4096
S = B // N_CORES         # samples per core = 2
CC = C // 128            # channel chunks = 2
G = 8                    # column blocks per sample (512 hw each)
NG = S * G               # global groups per core = 16
LOOKAHEAD = 3            # input-load groups ahead of compute


def build_program():
    nc = bacc.Bacc("TRN2", target_bir_lowering=False, debug=False)

    # block-major input: a_blk[s, g, p, cc, col] = a[s, cc*128+p, g*512+col]
    # so each (s, g) slab is one contiguous 256 KiB DMA whose element order
    # (p, cc, col) matches the SBUF destination AP -> 1 DMA + 1 semaphore
    # per group instead of 2.
    a_in = nc.dram_tensor("a_blk", [S, G, 128, CC, 512], BF16, kind="ExternalInput")
    # packed constants (see _make_in_maps): weights [128, 3*512] bf16 with
    # w_all[p, w*512 + cc*256 + o] = w^T[cc*128 + p, o]; biases [128, 4] f32
    # as columns (bq0, bq1, bk0, bk1); bvb [128, 512] f32 = bv tiled twice.
    w_in = nc.dram_tensor("w_all", [3, 128, 512], BF16, kind="ExternalInput")
    bqk_in = nc.dram_tensor("bqk", [128, 4], F32, kind="ExternalInput")
    bvb_in = nc.dram_tensor("bvb", [128, 512], F32, kind="ExternalInput")
    # block-major bf16 output, same layout as a_blk
    out_d = nc.dram_tensor("out", [S, G, 128, CC, 512], BF16, kind="ExternalOutput")

    with tile.TileContext(nc) as tc:
        with (
            tc.tile_pool(name="const", bufs=1) as const_pool,
            tc.tile_pool(name="xb", bufs=2) as xb_pool,
            tc.tile_pool(name="qk", bufs=2) as qk_pool,
            tc.tile_pool(name="vsb", bufs=2) as v_pool,
            tc.tile_pool(name="osb", bufs=8) as out_pool,
            tc.tile_pool(name="qkps", bufs=3, space=bass.MemorySpace.PSUM) as qkps_pool,
            tc.tile_pool(name="vps", bufs=2, space=bass.MemorySpace.PSUM) as vps_pool,
            tc.tile_pool(name="atp", bufs=1, space=bass.MemorySpace.PSUM) as atp_pool,
            tc.tile_pool(name="aop", bufs=2, space=bass.MemorySpace.PSUM) as aop_pool,
        ):
            # --- persistent constants: weights on the SP queue (stores only
            # start two iterations later, so no FIFO conflict); wq first so
            # the first projection can start as early as possible.
            w_all_sb = const_pool.tile([128, 3 * 512], BF16, tag="w_all")
            bqk_sb = const_pool.tile([128, 4], F32, tag="bqk")
            bvb_sb = const_pool.tile([128, 512], F32, tag="bvb")
            nc.sync.dma_start(w_all_sb[:, 0:512], w_in[0])
            w_sb = {"wq": w_all_sb[:, 0:512],
                    "wk": w_all_sb[:, 512:1024],
                    "wv": w_all_sb[:, 1024:1536]}
            bq_sb = bqk_sb[:, 0:2]
            bk_sb = bqk_sb[:, 2:4]

            # --- per-sample tiles, allocated lazily ---
            # xb holds both channel chunks: xb[p, cc*HW + col] = X[cc*128+p, col]
            xb_all = {}    # sv -> xb tile [128, CC*HW] bf16
            qt_all = {}    # sv -> [qt0, qt1]
            kt_all = {}
            v_all = {}     # sv -> v_sb [128, 32*C]  (V in [hw-part, c] layout)

            def ensure_xb(sv):
                if sv not in xb_all:
                    xb_all[sv] = xb_pool.tile([128, CC * HW], BF16, tag="xb",
                                              name="xb")
                return xb_all[sv]

            def emit_load(gi, eng):
                # one DMA per group: src slab is contiguous in DRAM and its
                # (p, cc, col) order matches the strided SBUF dst view
                sv, g = divmod(gi, G)
                xb = ensure_xb(sv)
                xbr = xb.rearrange("p (c n) -> p c n", c=CC)
                eng.dma_start(xbr[:, :, g * 512:(g + 1) * 512], a_in[sv, g])

            # prologue loads ride the two hardware DGE queues (idle during
            # startup, much faster to first-transfer than gpsimd's software
            # queue); g0 is split by channel chunk across BOTH queues so
            # its two halves transfer in parallel with wq. Later loads move
            # to gpsimd where the ~700ns enqueue cost is free.
            xb0 = ensure_xb(0)
            xb0r = xb0.rearrange("p (c n) -> p c n", c=CC)
            nc.scalar.dma_start(xb0r[:, 0, 0:512], a_in[0, 0, :, 0, :])
            nc.sync.dma_start(xb0r[:, 1, 0:512], a_in[0, 0, :, 1, :])
            nc.sync.dma_start(bqk_sb[:], bqk_in[:])
            nc.sync.dma_start(w_all_sb[:, 512:1024], w_in[1])
            nc.sync.dma_start(w_all_sb[:, 1024:1536], w_in[2])
            nc.sync.dma_start(bvb_sb[:], bvb_in[:])
            emit_load(1, nc.scalar)
            emit_load(2, nc.gpsimd)

            # at4 tiles hold 4 block-diagonal [128, 128] attnT matrices
            # ([128, 128]: h0 in [0:64, 0:64], h1 in [64:128, 64:128]).
            # The off-diagonal zeros are written ONCE here (after the
            # prologue loads, so they don't delay the first input block on
            # the gpsimd queue); extracts only ever write the diagonal
            # blocks, so the zeros persist across reuse and no per-group
            # memset is needed.
            at4_tiles = []
            for t in range(4):
                at4 = const_pool.tile([128, 512], BF16, tag=f"at4_{t}")
                at4r = at4.rearrange("p (j n) -> p j n", j=4)
                nc.gpsimd.memset(at4r[0:64, :, 64:128], 0.0)
                nc.gpsimd.memset(at4r[64:128, :, 0:64], 0.0)
                at4_tiles.append(at4)

            def emit_attn(ai):
                sv, g = divmod(ai, G)
                qt, kt = qt_all[sv], kt_all[sv]
                # attnT per h-pair j is built block-diagonal so the
                # attout matmul contracts over the full 128 partitions
                # (row-offset matmuls are broken in this stack).
                # Full-width [128,128] matmuls per (j, cc): both diagonal
                # blocks land in one pass (the off-diagonal garbage is
                # never extracted); 8 matmuls per group instead of 16
                # halves the PE issue slots.
                atps = atp_pool.tile([128, 512], F32, tag="atps",
                                     name="atps")
                for jj4 in range(4):
                    j = 4 * g + jj4
                    for cc in range(CC):
                        nc.tensor.matmul(
                            atps[:, jj4 * 128:(jj4 + 1) * 128],
                            kt[cc][:, j * 128:(j + 1) * 128],
                            qt[cc][:, j * 128:(j + 1) * 128],
                            start=(cc == 0),
                            stop=(cc == 1),
                        )
                # extracts split ACT/DVE: gpsimd cannot read PSUM, and
                # either engine alone would exceed the PE's per-iter pace
                at4 = at4_tiles[ai % 4]
                at4r = at4.rearrange("p (j n) -> p j n", j=4)
                atpsr = atps.rearrange("p (j n) -> p j n", j=4)
                nc.scalar.activation(at4r[0:64, :, 0:64],
                                     atpsr[0:64, :, 0:64], AF.Copy,
                                     bias=0.0)
                nc.vector.tensor_copy(at4r[64:128, :, 64:128],
                                      atpsr[64:128, :, 64:128])

            # --- pipelined main loop ---
            for i in range(NG + 2):
                # Stage A: projections for group i
                if i < NG:
                    sv, g = divmod(i, G)
                    xb = ensure_xb(sv)
                    if g == 0:
                        qt_all[sv] = [qk_pool.tile([128, HW], BF16,
                                                   tag=f"qt{oc}", name=f"qt{oc}")
                                      for oc in range(CC)]
                        kt_all[sv] = [qk_pool.tile([128, HW], BF16,
                                                   tag=f"kt{oc}", name=f"kt{oc}")
                                      for oc in range(CC)]
                        v_all[sv] = v_pool.tile([128, 32 * C], BF16,
                                                tag="v", name="v")
                    qt, kt, v_sb = qt_all[sv], kt_all[sv], v_all[sv]

                    # Q/K projections for column block g
                    for wname, bias_sb, dest in (("wq", bq_sb, qt),
                                                 ("wk", bk_sb, kt)):
                        for oc in range(CC):
                            ps = qkps_pool.tile([128, 512], F32, tag="ps",
                                                name="ps")
                            for cc in range(CC):
                                nc.tensor.matmul(
                                    ps[:],
                                    w_sb[wname][:, cc * C + oc * 128:
                                                cc * C + oc * 128 + 128],
                                    xb[:, cc * HW + g * 512:
                                       cc * HW + (g + 1) * 512],
                                    start=(cc == 0),
                                    stop=(cc == 1),
                                )
                            nc.scalar.activation(
                                dest[oc][:, g * 512:(g + 1) * 512],
                                ps[:],
                                AF.Identity,
                                bias=bias_sb[:, oc:oc + 1],
                            )
                    # V projection for hw chunks 4g..4g+3 (jj = 2g, 2g+1)
                    # v_sb[p, j*256 + c] = V[j*128 + p, c]
                    for jj in (2 * g, 2 * g + 1):
                        ps = vps_pool.tile([128, 512], F32, tag="vps",
                                           name="vps")
                        for u in range(2):
                            j = 2 * jj + u
                            for cc in range(CC):
                                nc.tensor.matmul(
                                    ps[:, u * C:(u + 1) * C],
                                    xb[:, cc * HW + j * 128:
                                       cc * HW + (j + 1) * 128],
                                    w_sb["wv"][:, cc * C:(cc + 1) * C],
                                    start=(cc == 0),
                                    stop=(cc == 1),
                                )
                        nc.vector.tensor_add(
                            v_sb[:, jj * 512:(jj + 1) * 512], ps[:], bvb_sb[:]
                        )

                # Stage B: attnT for group i-1 (the final group NG-1 is
                # instead emitted at the END of iteration NG-1, collapsing
                # the pipeline skew so the drain doesn't serialize on ACT)
                ai = i - 1
                if 0 <= ai < NG - 1:
                    emit_attn(ai)

                # Stage C: attout + residual + store for group i-2
                oi = i - 2
                if 0 <= oi:
                    sv, g = divmod(oi, G)
                    xb, v_sb = xb_all[sv], v_all[sv]
                    at4 = at4_tiles[oi % 4]
                    aop = [aop_pool.tile([128, 512], F32, tag="aop",
                                         name="aop") for _ in range(CC)]
                    for jj4 in range(4):
                        j = 4 * g + jj4
                        for cc in range(CC):
                            nc.tensor.matmul(
                                aop[cc][:, jj4 * 128:(jj4 + 1) * 128],
                                v_sb[:, j * C + cc * 128:
                                     j * C + (cc + 1) * 128],
                                at4[:, jj4 * 128:(jj4 + 1) * 128],
                                start=True,
                                stop=True,
                            )
                    # combined [128, 1024] residual tile -> one store DMA
                    osb = out_pool.tile([128, CC * 512], BF16, tag="osb",
                                        name="osb")
                    for cc in range(CC):
                        nc.vector.tensor_add(
                            osb[:, cc * 512:(cc + 1) * 512], aop[cc][:],
                            xb[:, cc * HW + g * 512:cc * HW + (g + 1) * 512]
                        )
                    nc.sync.dma_start(out_d[sv, g], osb[:])

                # input-load enqueues last: the extract above is the more
                # urgent work, and loads have 3 iterations of slack
                if i < NG and i + LOOKAHEAD < NG:
                    emit_load(i + LOOKAHEAD, nc.gpsimd)

                # collapse the skew for the final group: its attn matmuls
                # slot in right as its projection copies finish on ACT
                if i == NG - 1:
                    emit_attn(NG - 1)
    nc.compile()
    return nc


_NC_CACHE = None


def _get_program():
    global _NC_CACHE
    if _NC_CACHE is None:
        _NC_CACHE = build_program()
    return _NC_CACHE


def _make_in_maps(a, wq, bq, wk, bk, wv, bv):
    bf = ml_dtypes.bfloat16

    def pack_w(w):
        # w [c_out, c_in] -> SBUF view [128, cc*256 + c_out]
        w_t = np.asarray(w, np.float32).T.astype(bf)          # [c_in, c_out]
        return np.ascontiguousarray(
            w_t.reshape(2, 128, C).transpose(1, 0, 2).reshape(128, 2 * C))

    w_all = np.stack([pack_w(wq), pack_w(wk), pack_w(wv)], axis=0)
    bq_f = np.asarray(bq, np.float32)
    bk_f = np.asarray(bk, np.float32)
    bqk = np.ascontiguousarray(
        np.stack([bq_f[:128], bq_f[128:], bk_f[:128], bk_f[128:]], axis=1))
    bvb = np.tile(np.asarray(bv, np.float32).reshape(1, C), (128, 2))
    # block-major bf16 input: [B, G, 128, CC, 512]
    a_bf = np.asarray(a, np.float32).reshape(B, CC, 128, G, 512).astype(bf)
    a_blk = a_bf.transpose(0, 3, 2, 1, 4)
    in_maps = []
    for i in range(N_CORES):
        in_maps.append({
            "a_blk": np.ascontiguousarray(a_blk[i * S:(i + 1) * S]),
            "w_all": w_all, "bqk": bqk, "bvb": bvb,
        })
    return in_maps


def _unpack_out(out_blk):
    # [B, G, 128, CC, 512] bf16 -> [B, C, H, W] f32
    out = np.asarray(out_blk).transpose(0, 3, 2, 1, 4).astype(np.float32)
    return np.ascontiguousarray(out.reshape(B, C, H, W))


def run(a, wq, bq, wk, bk, wv, bv, trace=False, **trace_kw):
    nc = _get_program()
    in_maps = _make_in_maps(a, wq, bq, wk, bk, wv, bv)
    res = run_bass_kernel_spmd(nc, in_maps, list(range(N_CORES)), trace=trace,
                               **trace_kw)
    out = np.concatenate([np.asarray(r["out"]) for r in res.results], axis=0)
    return _unpack_out(out), res


_JIT_CACHE = None


def _get_sharded():
    """Build (once) a jitted shard_map dispatch of the NEFF across 8 cores.

    run_bass_kernel_spmd re-traces and re-builds its jit wrapper on every
    call; caching the jitted callable makes repeat kernel() invocations
    dispatch directly.
    """
    global _JIT_CACHE
    if _JIT_CACHE is None:
        import jax
        from jax.sharding import Mesh, PartitionSpec
        from jax.experimental.shard_map import shard_map
        from concourse import bass2jax

        nc = _get_program()
        bass2jax.install_neuronx_cc_hook()
        partition_name = (nc.partition_id_tensor.name
                          if nc.partition_id_tensor else None)
        in_names, out_names, out_avals, zero_outs = [], [], [], []
        for alloc in nc.m.functions[0].allocations:
            if not isinstance(alloc, mybir.MemoryLocationSet):
                continue
            name = alloc.memorylocations[0].name
            if alloc.kind == "ExternalInput":
                if name != partition_name:
                    in_names.append(name)
            elif alloc.kind == "ExternalOutput":
                shape = tuple(alloc.tensor_shape)
                dtype = mybir.dt.np(alloc.dtype)
                out_avals.append(jax.core.ShapedArray(shape, dtype))
                out_names.append(name)
                zero_outs.append(np.zeros(shape, dtype))
        n_params = len(in_names)
        all_in_names = in_names + out_names + (
            [partition_name] if partition_name else [])

        def _body(*args):
            operands = list(args)
            if partition_name is not None:
                operands.append(bass2jax.partition_id_tensor())
            outs = bass2jax._bass_exec_p.bind(
                *operands,
                out_avals=tuple(out_avals),
                in_names=tuple(all_in_names),
                out_names=tuple(out_names),
                lowering_input_output_aliases=(),
                sim_require_finite=True,
                sim_require_nnan=True,
                nc=nc,
            )
            return tuple(outs)

        devices = jax.devices()[:N_CORES]
        mesh = Mesh(np.asarray(devices), ("core",))
        n_outs = len(out_names)
        sharded = jax.jit(
            shard_map(_body, mesh=mesh,
                      in_specs=(PartitionSpec("core"),) * (n_params + n_outs),
                      out_specs=(PartitionSpec("core"),) * n_outs,
                      check_rep=False),
            keep_unused=True,
        )
        _JIT_CACHE = (sharded, in_names, out_names, zero_outs)
    return _JIT_CACHE


def kernel(a, wq, bq, wk, bk, wv, bv):
    sharded, in_names, out_names, zero_outs = _get_sharded()
    in_maps = _make_in_maps(a, wq, bq, wk, bk, wv, bv)
    concat_in = [np.concatenate([m[nm] for m in in_maps], axis=0)
                 for nm in in_names]
    concat_zeros = [np.zeros((N_CORES * z.shape[0], *z.shape[1:]), z.dtype)
                    for z in zero_outs]
    outs = sharded(*concat_in, *concat_zeros)
    out_blk = np.asarray(outs[out_names.index("out")])
    return _unpack_out(out_blk)


# revision 32
# speedup vs baseline: 1.0167x; 1.0167x over previous
"""Trainium2 Bass kernel for nn_MultiHeadAttn (unnormalized spatial attention).

Reference computation (per sample s of B=16):
    X = a[s]               # [C=256, HW=4096]  (H=64 rows of W=64)
    QT = wq @ X + bq       # [C, HW]   (q channels on rows)
    KT = wk @ X + bk
    V  = (wv @ X + bv).T   # [HW, C]   (hw on rows)
    per h: attnT_h = K_h @ Q_h^T        # [W, W]  == (Q_h K_h^T)^T
           attoutT_h = V_h^T @ attnT_h  # [C, W]
    out[s] = a[s] + attoutT (reassembled [C, HW])

Sharding: data-parallel over batch, 2 samples per core on 8 cores.
All matmuls in bf16 (fp32 PSUM accumulation); residual in fp32 PSUM +
bf16 operand, stored as bf16 and widened to f32 on the host.

Schedule: 16 global groups (2 samples x 8 column blocks of 512 hw
positions) run through a 2-stage software pipeline so the PE never
waits on the ACT engine:
    iteration i: projections(i) | QK^T(i-1) + extract | attout(i-2)
Input loads ride the ACT hardware DGE queue, stores the SP queue, so
sample 1's loads never sit behind sample 0's output stores.
"""

import numpy as np
import ml_dtypes

import concourse.bass as bass
import concourse.mybir as mybir
import concourse.tile as tile
from concourse import bacc
from concourse.bass_utils import run_bass_kernel_spmd

BF16 = mybir.dt.bfloat16
F32 = mybir.dt.float32
AF = mybir.ActivationFunctionType

N_CORES = 8
B, C, H, W = 16, 256, 64, 64
HW = H * W               # 4096
S = B // N_CORES         # samples per core = 2
CC = C // 128            # channel chunks = 2
G = 8                    # column blocks per sample (512 hw each)
NG = S * G               # global groups per core = 16
LOOKAHEAD = 3            # input-load groups ahead of compute


def build_program():
    nc = bacc.Bacc("TRN2", target_bir_lowering=False, debug=False)

    # block-major input: a_blk[s, g, p, cc, col] = a[s, cc*128+p, g*512+col]
    # so each (s, g) slab is one contiguous 256 KiB DMA whose element order
    # (p, cc, col) matches the SBUF destination AP -> 1 DMA + 1 semaphore
    # per group instead of 2.
    a_in = nc.dram_tensor("a_blk", [S, G, 128, CC, 512], BF16, kind="ExternalInput")
    # packed constants (see _make_in_maps): weights [128, 3*512] bf16 with
    # w_all[p, w*512 + cc*256 + o] = w^T[cc*128 + p, o]; biases [128, 4] f32
    # as columns (bq0, bq1, bk0, bk1); bvb [128, 512] f32 = bv tiled twice.
    w_in = nc.dram_tensor("w_all", [3, 128, 512], BF16, kind="ExternalInput")
    bqk_in = nc.dram_tensor("bqk", [128, 4], F32, kind="ExternalInput")
    bvb_in = nc.dram_tensor("bvb", [128, 512], F32, kind="ExternalInput")
    # block-major bf16 output, same layout as a_blk
    out_d = nc.dram_tensor("out", [S, G, 128, CC, 512], BF16, kind="ExternalOutput")

    with tile.TileContext(nc) as tc:
        with (
            tc.tile_pool(name="const", bufs=1) as const_pool,
            tc.tile_pool(name="xb", bufs=2) as xb_pool,
            tc.tile_pool(name="qk", bufs=2) as qk_pool,
            tc.tile_pool(name="vsb", bufs=2) as v_pool,
            tc.tile_pool(name="osb", bufs=8) as out_pool,
            tc.tile_pool(name="qkps", bufs=3, space=bass.MemorySpace.PSUM) as qkps_pool,
            tc.tile_pool(name="vps", bufs=2, space=bass.MemorySpace.PSUM) as vps_pool,
            tc.tile_pool(name="atp", bufs=1, space=bass.MemorySpace.PSUM) as atp_pool,
            tc.tile_pool(name="aop", bufs=2, space=bass.MemorySpace.PSUM) as aop_pool,
        ):
            # --- persistent constants: weights on the SP queue (stores only
            # start two iterations later, so no FIFO conflict); wq first so
            # the first projection can start as early as possible.
            w_all_sb = const_pool.tile([128, 3 * 512], BF16, tag="w_all")
            bqk_sb = const_pool.tile([128, 4], F32, tag="bqk")
            bvb_sb = const_pool.tile([128, 512], F32, tag="bvb")
            nc.sync.dma_start(w_all_sb[:, 0:512], w_in[0])
            w_sb = {"wq": w_all_sb[:, 0:512],
                    "wk": w_all_sb[:, 512:1024],
                    "wv": w_all_sb[:, 1024:1536]}
            bq_sb = bqk_sb[:, 0:2]
            bk_sb = bqk_sb[:, 2:4]

            # --- per-sample tiles, allocated lazily ---
            # xb holds both channel chunks: xb[p, cc*HW + col] = X[cc*128+p, col]
            xb_all = {}    # sv -> xb tile [128, CC*HW] bf16
            qt_all = {}    # sv -> [qt0, qt1]
            kt_all = {}
            v_all = {}     # sv -> v_sb [128, 32*C]  (V in [hw-part, c] layout)

            def ensure_xb(sv):
                if sv not in xb_all:
                    xb_all[sv] = xb_pool.tile([128, CC * HW], BF16, tag="xb",
                                              name="xb")
                return xb_all[sv]

            def emit_load(gi, eng):
                # one DMA per group: src slab is contiguous in DRAM and its
                # (p, cc, col) order matches the strided SBUF dst view
                sv, g = divmod(gi, G)
                xb = ensure_xb(sv)
                xbr = xb.rearrange("p (c n) -> p c n", c=CC)
                eng.dma_start(xbr[:, :, g * 512:(g + 1) * 512], a_in[sv, g])

            # prologue loads ride the two hardware DGE queues (idle during
            # startup, much faster to first-transfer than gpsimd's software
            # queue); g0 is split by channel chunk across BOTH queues so
            # its two halves transfer in parallel with wq. Later loads move
            # to gpsimd where the ~700ns enqueue cost is free.
            xb0 = ensure_xb(0)
            xb0r = xb0.rearrange("p (c n) -> p c n", c=CC)
            nc.scalar.dma_start(xb0r[:, 0, 0:512], a_in[0, 0, :, 0, :])
            nc.scalar.dma_start(xb0r[:, 1, 0:512], a_in[0, 0, :, 1, :])
            nc.sync.dma_start(bqk_sb[:], bqk_in[:])
            nc.sync.dma_start(w_all_sb[:, 512:1024], w_in[1])
            nc.sync.dma_start(w_all_sb[:, 1024:1536], w_in[2])
            nc.sync.dma_start(bvb_sb[:], bvb_in[:])
            emit_load(1, nc.scalar)
            emit_load(2, nc.gpsimd)

            # at4 tiles hold 4 block-diagonal [128, 128] attnT matrices
            # ([128, 128]: h0 in [0:64, 0:64], h1 in [64:128, 64:128]).
            # The off-diagonal zeros are written ONCE here (after the
            # prologue loads, so they don't delay the first input block on
            # the gpsimd queue); extracts only ever write the diagonal
            # blocks, so the zeros persist across reuse and no per-group
            # memset is needed.
            at4_tiles = []
            for t in range(4):
                at4 = const_pool.tile([128, 512], BF16, tag=f"at4_{t}")
                at4r = at4.rearrange("p (j n) -> p j n", j=4)
                nc.gpsimd.memset(at4r[0:64, :, 64:128], 0.0)
                nc.gpsimd.memset(at4r[64:128, :, 0:64], 0.0)
                at4_tiles.append(at4)

            def emit_attn(ai):
                sv, g = divmod(ai, G)
                qt, kt = qt_all[sv], kt_all[sv]
                # attnT per h-pair j is built block-diagonal so the
                # attout matmul contracts over the full 128 partitions
                # (row-offset matmuls are broken in this stack).
                # Full-width [128,128] matmuls per (j, cc): both diagonal
                # blocks land in one pass (the off-diagonal garbage is
                # never extracted); 8 matmuls per group instead of 16
                # halves the PE issue slots.
                atps = atp_pool.tile([128, 512], F32, tag="atps",
                                     name="atps")
                for jj4 in range(4):
                    j = 4 * g + jj4
                    for cc in range(CC):
                        nc.tensor.matmul(
                            atps[:, jj4 * 128:(jj4 + 1) * 128],
                            kt[cc][:, j * 128:(j + 1) * 128],
                            qt[cc][:, j * 128:(j + 1) * 128],
                            start=(cc == 0),
                            stop=(cc == 1),
                        )
                # extracts split ACT/DVE: gpsimd cannot read PSUM, and
                # either engine alone would exceed the PE's per-iter pace
                at4 = at4_tiles[ai % 4]
                at4r = at4.rearrange("p (j n) -> p j n", j=4)
                atpsr = atps.rearrange("p (j n) -> p j n", j=4)
                nc.scalar.activation(at4r[0:64, :, 0:64],
                                     atpsr[0:64, :, 0:64], AF.Copy,
                                     bias=0.0)
                nc.vector.tensor_copy(at4r[64:128, :, 64:128],
                                      atpsr[64:128, :, 64:128])

            # --- pipelined main loop ---
            for i in range(NG + 2):
                # Stage A: projections for group i
                if i < NG:
                    sv, g = divmod(i, G)
                    xb = ensure_xb(sv)
                    if g == 0:
                        qt_all[sv] = [qk_pool.tile([128, HW], BF16,
                                                   tag=f"qt{oc}", name=f"qt{oc}")
                                      for oc in range(CC)]
                        kt_all[sv] = [qk_pool.tile([128, HW], BF16,
                                                   tag=f"kt{oc}", name=f"kt{oc}")
                                      for oc in range(CC)]
                        v_all[sv] = v_pool.tile([128, 32 * C], BF16,
                                                tag="v", name="v")
                    qt, kt, v_sb = qt_all[sv], kt_all[sv], v_all[sv]

                    # Q/K projections for column block g
                    for wname, bias_sb, dest in (("wq", bq_sb, qt),
                                                 ("wk", bk_sb, kt)):
                        for oc in range(CC):
                            ps = qkps_pool.tile([128, 512], F32, tag="ps",
                                                name="ps")
                            for cc in range(CC):
                                nc.tensor.matmul(
                                    ps[:],
                                    w_sb[wname][:, cc * C + oc * 128:
                                                cc * C + oc * 128 + 128],
                                    xb[:, cc * HW + g * 512:
                                       cc * HW + (g + 1) * 512],
                                    start=(cc == 0),
                                    stop=(cc == 1),
                                )
                            nc.scalar.activation(
                                dest[oc][:, g * 512:(g + 1) * 512],
                                ps[:],
                                AF.Identity,
                                bias=bias_sb[:, oc:oc + 1],
                            )
                    # V projection for hw chunks 4g..4g+3 (jj = 2g, 2g+1)
                    # v_sb[p, j*256 + c] = V[j*128 + p, c]
                    for jj in (2 * g, 2 * g + 1):
                        ps = vps_pool.tile([128, 512], F32, tag="vps",
                                           name="vps")
                        for u in range(2):
                            j = 2 * jj + u
                            for cc in range(CC):
                                nc.tensor.matmul(
                                    ps[:, u * C:(u + 1) * C],
                                    xb[:, cc * HW + j * 128:
                                       cc * HW + (j + 1) * 128],
                                    w_sb["wv"][:, cc * C:(cc + 1) * C],
                                    start=(cc == 0),
                                    stop=(cc == 1),
                                )
                        nc.vector.tensor_add(
                            v_sb[:, jj * 512:(jj + 1) * 512], ps[:], bvb_sb[:]
                        )

                # Stage B: attnT for group i-1 (the final group NG-1 is
                # instead emitted at the END of iteration NG-1, collapsing
                # the pipeline skew so the drain doesn't serialize on ACT)
                ai = i - 1
                if 0 <= ai < NG - 1:
                    emit_attn(ai)

                # Stage C: attout + residual + store for group i-2
                oi = i - 2
                if 0 <= oi:
                    sv, g = divmod(oi, G)
                    xb, v_sb = xb_all[sv], v_all[sv]
                    at4 = at4_tiles[oi % 4]
                    aop = [aop_pool.tile([128, 512], F32, tag="aop",
                                         name="aop") for _ in range(CC)]
                    for jj4 in range(4):
                        j = 4 * g + jj4
                        for cc in range(CC):
                            nc.tensor.matmul(
                                aop[cc][:, jj4 * 128:(jj4 + 1) * 128],
                                v_sb[:, j * C + cc * 128:
                                     j * C + (cc + 1) * 128],
                                at4[:, jj4 * 128:(jj4 + 1) * 128],
                                start=True,
                                stop=True,
                            )
                    # combined [128, 1024] residual tile -> one store DMA
                    osb = out_pool.tile([128, CC * 512], BF16, tag="osb",
                                        name="osb")
                    for cc in range(CC):
                        nc.vector.tensor_add(
                            osb[:, cc * 512:(cc + 1) * 512], aop[cc][:],
                            xb[:, cc * HW + g * 512:cc * HW + (g + 1) * 512]
                        )
                    nc.sync.dma_start(out_d[sv, g], osb[:])

                # input-load enqueues last: the extract above is the more
                # urgent work, and loads have 3 iterations of slack
                if i < NG and i + LOOKAHEAD < NG:
                    emit_load(i + LOOKAHEAD, nc.gpsimd)

                # collapse the skew for the final group: its attn matmuls
                # slot in right as its projection copies finish on ACT
                if i == NG - 1:
                    emit_attn(NG - 1)
    nc.compile()
    return nc


_NC_CACHE = None


def _get_program():
    global _NC_CACHE
    if _NC_CACHE is None:
        _NC_CACHE = build_program()
    return _NC_CACHE


def _make_in_maps(a, wq, bq, wk, bk, wv, bv):
    bf = ml_dtypes.bfloat16

    def pack_w(w):
        # w [c_out, c_in] -> SBUF view [128, cc*256 + c_out]
        w_t = np.asarray(w, np.float32).T.astype(bf)          # [c_in, c_out]
        return np.ascontiguousarray(
            w_t.reshape(2, 128, C).transpose(1, 0, 2).reshape(128, 2 * C))

    w_all = np.stack([pack_w(wq), pack_w(wk), pack_w(wv)], axis=0)
    bq_f = np.asarray(bq, np.float32)
    bk_f = np.asarray(bk, np.float32)
    bqk = np.ascontiguousarray(
        np.stack([bq_f[:128], bq_f[128:], bk_f[:128], bk_f[128:]], axis=1))
    bvb = np.tile(np.asarray(bv, np.float32).reshape(1, C), (128, 2))
    # block-major bf16 input: [B, G, 128, CC, 512]
    a_bf = np.asarray(a, np.float32).reshape(B, CC, 128, G, 512).astype(bf)
    a_blk = a_bf.transpose(0, 3, 2, 1, 4)
    in_maps = []
    for i in range(N_CORES):
        in_maps.append({
            "a_blk": np.ascontiguousarray(a_blk[i * S:(i + 1) * S]),
            "w_all": w_all, "bqk": bqk, "bvb": bvb,
        })
    return in_maps


def _unpack_out(out_blk):
    # [B, G, 128, CC, 512] bf16 -> [B, C, H, W] f32
    out = np.asarray(out_blk).transpose(0, 3, 2, 1, 4).astype(np.float32)
    return np.ascontiguousarray(out.reshape(B, C, H, W))


def run(a, wq, bq, wk, bk, wv, bv, trace=False, **trace_kw):
    nc = _get_program()
    in_maps = _make_in_maps(a, wq, bq, wk, bk, wv, bv)
    res = run_bass_kernel_spmd(nc, in_maps, list(range(N_CORES)), trace=trace,
                               **trace_kw)
    out = np.concatenate([np.asarray(r["out"]) for r in res.results], axis=0)
    return _unpack_out(out), res


_JIT_CACHE = None


def _get_sharded():
    """Build (once) a jitted shard_map dispatch of the NEFF across 8 cores.

    run_bass_kernel_spmd re-traces and re-builds its jit wrapper on every
    call; caching the jitted callable makes repeat kernel() invocations
    dispatch directly.
    """
    global _JIT_CACHE
    if _JIT_CACHE is None:
        import jax
        from jax.sharding import Mesh, PartitionSpec
        from jax.experimental.shard_map import shard_map
        from concourse import bass2jax

        nc = _get_program()
        bass2jax.install_neuronx_cc_hook()
        partition_name = (nc.partition_id_tensor.name
                          if nc.partition_id_tensor else None)
        in_names, out_names, out_avals, zero_outs = [], [], [], []
        for alloc in nc.m.functions[0].allocations:
            if not isinstance(alloc, mybir.MemoryLocationSet):
                continue
            name = alloc.memorylocations[0].name
            if alloc.kind == "ExternalInput":
                if name != partition_name:
                    in_names.append(name)
            elif alloc.kind == "ExternalOutput":
                shape = tuple(alloc.tensor_shape)
                dtype = mybir.dt.np(alloc.dtype)
                out_avals.append(jax.core.ShapedArray(shape, dtype))
                out_names.append(name)
                zero_outs.append(np.zeros(shape, dtype))
        n_params = len(in_names)
        all_in_names = in_names + out_names + (
            [partition_name] if partition_name else [])

        def _body(*args):
            operands = list(args)
            if partition_name is not None:
                operands.append(bass2jax.partition_id_tensor())
            outs = bass2jax._bass_exec_p.bind(
                *operands,
                out_avals=tuple(out_avals),
                in_names=tuple(all_in_names),
                out_names=tuple(out_names),
                lowering_input_output_aliases=(),
                sim_require_finite=True,
                sim_require_nnan=True,
                nc=nc,
            )
            return tuple(outs)

        devices = jax.devices()[:N_CORES]
        mesh = Mesh(np.asarray(devices), ("core",))
        n_outs = len(out_names)
        sharded = jax.jit(
            shard_map(_body, mesh=mesh,
                      in_specs=(PartitionSpec("core"),) * (n_params + n_outs),
                      out_specs=(PartitionSpec("core"),) * n_outs,
                      check_rep=False),
            keep_unused=True,
        )
        _JIT_CACHE = (sharded, in_names, out_names, zero_outs)
    return _JIT_CACHE


def kernel(a, wq, bq, wk, bk, wv, bv):
    sharded, in_names, out_names, zero_outs = _get_sharded()
    in_maps = _make_in_maps(a, wq, bq, wk, bk, wv, bv)
    concat_in = [np.concatenate([m[nm] for m in in_maps], axis=0)
                 for nm in in_names]
    concat_zeros = [np.zeros((N_CORES * z.shape[0], *z.shape[1:]), z.dtype)
                    for z in zero_outs]
    outs = sharded(*concat_in, *concat_zeros)
    out_blk = np.asarray(outs[out_names.index("out")])
    return _unpack_out(out_blk)
